# revision 18
# baseline (speedup 1.0000x reference)
"""Trainium2 Bass kernel for batched multi-head softmax attention.

Problem: q,k,v [B=4, H=16, N=2048, D=64] fp32.
  out = softmax(q @ k^T / sqrt(D)) @ v   (per b,h)

Sharding: B*H = 64 head-slices, 8 per core across 8 cores. Each core
computes full attention for its 8 heads independently (no collectives).

Per-head algorithm on one core (i = query index, j = key index):
  - Load Q,K natural f32, cast to bf16 (DVE). K is pre-scaled by
    FE_MUL=128*log2(e)/8 during the cast so the QK product lands
    already in the fast-exp domain.
  - PE-transpose Q,K. The transpose of a [128, 2*64] slice lands even
    blocks on partitions 0-63 and odd blocks on partitions 64-127:
    for Q that IS the split-stream layout needed for PE row tiling
    (no duplication); for K a second GPSIMD-built tile with swapped
    halves provides each j-block on both partition halves.
  - V' = [V | 1] per j-block, bf16: the 65th PV output row accumulates
    sum_j e = the softmax denominator for free.
  - Flash-style loop: for each i-half icp (1024), for each j-block jb:
      row-tiled QK: tile (0,0) computes S'^T[j, even i] from partitions
      0-63, tile (64,0) computes S'^T[j, odd i] from partitions 64-127,
      concurrently (K=64 contraction only needs half the PE rows).
      S' = FE_MUL/8 * S_raw (pre-scaled via K).
      E: even half exact exp on ACT (scale folds the pre-scale back);
      odd half Schraudolph fast-exp on DVE, now a single tensor_scalar
      ADD: int16(S' + 16249) is the bf16 bit pattern of ~exp(S_raw/8)
      (~1.8% rms; inside the 2e-2 budget, partially cancelling in the
      softmax normalization).
      O^T += V'[jb]^T . E (bf16 accumulating matmuls, 2x512 moving;
      the ISA caps moving free size at 512).
  - O^T is PE-transposed back and scaled by 1/denominator (DVE).
"""

import numpy as np
from collections import deque

B, H, N, D = 4, 16, 2048, 64
NCORES = 8
HPC = (B * H) // NCORES  # heads per core = 8
NB = N // 128  # 16 j-blocks / i-blocks of 128
SCALE = float(D) ** -0.5

# Schraudolph fast-exp: int16 bf16-bit-pattern of exp(s) for s=S_raw/8:
#   bits = trunc(s * 128*log2(e) + (127*128 - C + 0.5))
# C=7.5 minimizes rms relative error (~1.78%); +0.5 converts the
# engine's truncating f32->i16 store into round-to-nearest. The
# multiply is folded into the K bf16 cast (FE_MUL), so the DVE op in
# the inner loop is a single ADD.
FE_MUL = 128.0 * 1.4426950408889634 * SCALE
FE_ADD = 127.0 * 128.0 - 7.5 + 0.5
ACT_SCALE = SCALE / FE_MUL  # activation sees S'=FE_MUL*S_raw; wants S_raw/8
ACT_COLS = 576  # exp split: ACT does [0:ACT_COLS), DVE fast-exp the rest

_cache = {}


def _build(hpc=HPC, qk_dt="bfloat16", pv_dt="bfloat16"):
    # note: walrus's --enable-ldw-opt (redundant LDWEIGHTS elimination)
    # would remove the repeated PV stationary loads, but the pass crashes
    # codegen (visitInstLdweights unhandled exception) on this kernel.
    import concourse.bacc as bacc
    import concourse.tile as tile
    from concourse import mybir
    from concourse.masks import make_identity

    f32 = mybir.dt.float32
    i16 = mybir.dt.int16
    qkd = getattr(mybir.dt, qk_dt)
    pvd = getattr(mybir.dt, pv_dt)
    EXP = mybir.ActivationFunctionType.Exp

    nc = bacc.Bacc("TRN2", target_bir_lowering=False, debug=False)
    q = nc.dram_tensor("q", [hpc, N, D], f32, kind="ExternalInput").ap()
    k = nc.dram_tensor("k", [hpc, N, D], f32, kind="ExternalInput").ap()
    v = nc.dram_tensor("v", [hpc, N, D], f32, kind="ExternalInput").ap()
    out = nc.dram_tensor("out", [hpc, N, D], f32, kind="ExternalOutput").ap()

    with tile.TileContext(nc) as tc:
        with (
            tc.tile_pool(name="consts", bufs=1) as consts,
            tc.tile_pool(name="stage", bufs=2) as stage,
            tc.tile_pool(name="qkt", bufs=2) as qkt,
            tc.tile_pool(name="epool", bufs=3) as epool,
            tc.tile_pool(name="osb", bufs=2) as osb,
            tc.tile_pool(name="outp", bufs=2) as outp,
            tc.tile_pool(name="stp", bufs=2, space="PSUM") as stp,
            tc.tile_pool(name="opsp", bufs=1, space="PSUM") as opsp,
            tc.tile_pool(name="tpp", bufs=2, space="PSUM") as tpp,
        ):
            warm_w = consts.tile([128, 128], mybir.dt.bfloat16)
            nc.vector.memset(warm_w[:], 0.0)
            # Preload the ACT exp table set before the first real exp
            dummy_e = consts.tile([128, 1], f32)
            nc.scalar.activation(dummy_e[:], warm_w[:, 0:1], EXP)

            ident = consts.tile([128, 128], f32)
            identb = consts.tile([128, 128], qkd)

            def emit_consts():
                # emitted after head 0's DMAs are queued so the loads start
                # as early as possible
                make_identity(nc, ident[:])
                nc.vector.tensor_copy(identb[:], ident[:])

            # PE warmup: real matmuls keep the HAM clock gate at 8/8
            def warm_burst(n):
                warm = stp.tile([128, 128], f32, tag="st", name="warm")
                for _ in range(n):
                    nc.tensor.matmul(
                        warm[:],
                        warm_w[:, 0:128],
                        warm_w[:, 0:128],
                        start=True,
                        stop=True,
                    )

            fillers = deque()

            def run_fillers(jb, njb=16):
                left = njb - jb
                kk = (len(fillers) + left - 1) // left if left > 0 else len(fillers)
                for _ in range(min(kk, len(fillers))):
                    fillers.popleft()()

            def flush_fillers():
                while fillers:
                    fillers.popleft()()

            def emit_loads(h):
                """DMA + casts for head h. Returns tiles."""
                q_nat = stage.tile([128, NB * D], f32, tag="q_nat", name="q_nat")
                nc.sync.dma_start(
                    out=q_nat.rearrange("p (b d) -> p b d", b=NB),
                    in_=q[h].rearrange("(b p) d -> p b d", p=128),
                )
                k_nat = stage.tile([128, NB * D], f32, tag="k_nat", name="k_nat")
                nc.sync.dma_start(
                    out=k_nat.rearrange("p (b d) -> p b d", b=NB),
                    in_=k[h].rearrange("(b p) d -> p b d", p=128),
                )
                q_bf = stage.tile([128, NB * D], qkd, tag="q_bf", name="q_bf")
                nc.vector.tensor_copy(q_bf[:], q_nat[:])
                # K cast folds the fast-exp multiplier
                k_bf = stage.tile([128, NB * D], qkd, tag="k_bf", name="k_bf")
                nc.vector.tensor_scalar_mul(k_bf[:], k_nat[:], FE_MUL)
                v_stage = stage.tile(
                    [128, NB * (D + 1)], f32, tag="v_stage", name="v_stage"
                )
                nc.sync.dma_start(
                    out=v_stage.rearrange("p (b e) -> p b e", b=NB)[:, :, 0:D],
                    in_=v[h].rearrange("(b p) d -> p b d", p=128),
                )
                nc.gpsimd.memset(
                    v_stage.rearrange("p (b e) -> p b e", b=NB)[:, :, D : D + 1], 1.0
                )
                v_r = stage.tile([128, NB * (D + 1)], pvd, tag="v_r", name="v_r")
                nc.vector.tensor_copy(v_r[:], v_stage[:])
                return q_bf, k_bf, v_r

            def queue_transposes(q_bf, k_bf, prologue=False):
                """Build Q^T [128,1024] (even i-blocks on parts 0-63, odd on
                64-127), K^T raw [128,1024] (even j-blocks low / odd high)
                and K^T swapped [128,1024] (odd low / even high)."""
                qtr = qkt.tile([128, N // 2], qkd, tag="qt", name="qtr")
                kraw = qkt.tile([128, N // 2], qkd, tag="kr", name="kraw")
                ksw = qkt.tile([128, N // 2], qkd, tag="ks", name="ksw")
                idx = 0
                # interleave Q and K so a partial prologue flush covers the
                # first jb iterations of both operands
                for t2 in range(NB // 2):
                    for src, dst in ((q_bf, qtr), (k_bf, kraw)):
                        idx += 1

                        def tr(src=src, dst=dst, t2=t2, idx=idx):
                            tp = (
                                stp.tile([128, 128], qkd, tag="st", name="tp")
                                if (prologue and idx % 2)
                                else tpp.tile([128, 128], qkd, tag="tp", name="tp")
                            )
                            nc.tensor.matmul(
                                tp[:],
                                src[:, t2 * 2 * D : (t2 * 2 + 2) * D],
                                identb[:, 0:128],
                                is_transpose=True,
                            )
                            cols = slice(t2 * 128, (t2 + 1) * 128)
                            nc.vector.tensor_copy(dst[:, cols], tp[:])
                            if dst is kraw:
                                # swapped-halves duplicate for the row tiles
                                if prologue:
                                    nc.scalar.copy(ksw[0:64, cols], tp[64:128, :])
                                    nc.scalar.copy(ksw[64:128, cols], tp[0:64, :])
                                else:
                                    nc.gpsimd.tensor_copy(
                                        ksw[0:64, cols], dst[64:128, cols]
                                    )
                                    nc.gpsimd.tensor_copy(
                                        ksw[64:128, cols], dst[0:64, cols]
                                    )

                        fillers.append(tr)
                return qtr, kraw, ksw

            def queue_norm(o_ps, icp, out_sb):
                """Copy O^T out of PSUM (frees the accumulator), queue the
                transpose+normalize steps as fillers. Column c of o_ps is
                query i = icp*1024 + 256*((c%512)>>7) + 128*(c>=512) + (c&127).
                The 4 t-blocks of one s-half transpose into a single PSUM
                tile, then one reciprocal + one stride-0-broadcast multiply
                normalizes all 4 blocks (out block index = icp*8 + 2t + s)."""
                o_sb = osb.tile([65, 1024], f32, tag="o_sb", name="o_sb")
                nc.scalar.copy(o_sb[:, 0:512], o_ps[0:65, 0:512])
                nc.vector.tensor_copy(o_sb[:, 512:1024], o_ps[0:65, 512:1024])
                for s in range(2):
                    pt4 = tpp.tile([128, 4 * 65], f32, tag="tp", name="pt4")
                    for t in range(4):

                        def step_t(s=s, t=t, o_sb=o_sb, pt4=pt4):
                            c0 = s * 512 + t * 128
                            nc.tensor.matmul(
                                pt4[:, t * 65 : (t + 1) * 65],
                                o_sb[:, c0 : c0 + 128],
                                ident[0:65, 0:65],
                                is_transpose=True,
                            )

                        fillers.append(step_t)

                    def step_norm(s=s, icp=icp, out_sb=out_sb, pt4=pt4):
                        ptv = pt4[:].rearrange("p (t c) -> p t c", c=65)
                        rec = osb.tile([128, 4], f32, tag="rec", name="rec")
                        nc.vector.reciprocal(rec[:], ptv[:, :, 64:65].squeeze(-1))
                        ov = out_sb.rearrange(
                            "p (a t s d) -> p a t s d", a=2, t=4, s=2
                        )
                        nc.vector.tensor_tensor(
                            ov[:, icp, :, s, :],
                            ptv[:, :, 0:64],
                            rec[:].unsqueeze(-1).broadcast_to([128, 4, D]),
                            mybir.AluOpType.mult,
                        )

                    fillers.append(step_norm)

            # ---------- prologue: head 0 ----------
            q_bf, k_bf, v_r = emit_loads(0)
            emit_consts()
            qtr, kraw, ksw = queue_transposes(q_bf, k_bf, prologue=True)
            warm_burst(24)
            # flush only the transposes the first jb iterations need (Q/K
            # t2 0-3, interleaved); the rest drain as in-loop fillers
            for _ in range(8):
                fillers.popleft()()
            nxt = {}

            for h in range(hpc):
                out_sb = outp.tile([128, NB * D], f32, tag="out_sb", name="out_sb")

                for icp in range(2):
                    if icp == 1 and h + 1 < hpc:
                        nq_bf, nk_bf, nv_r = emit_loads(h + 1)
                        nqtr, nkraw, nksw = queue_transposes(nq_bf, nk_bf)
                        nxt = {"v_r": nv_r, "qtr": nqtr, "kraw": nkraw, "ksw": nksw}

                    o_ps = opsp.tile([128, 1024], f32, tag="o", name="o_ps")
                    sts = {}

                    def emit_qk(jb, icp=icp, sts=sts, qtr=qtr, kraw=kraw, ksw=ksw):
                        st = stp.tile([128, 1024], f32, tag="st", name="st")
                        sts[jb] = st
                        m = jb // 2
                        cols = slice(m * 128, (m + 1) * 128)
                        if jb % 2 == 0:
                            lo, hi = kraw[0:64, cols], ksw[64:128, cols]
                        else:
                            lo, hi = ksw[0:64, cols], kraw[64:128, cols]
                        nc.tensor.matmul(
                            st[:, 0:512],
                            lo,
                            qtr[0:64, icp * 512 : (icp + 1) * 512],
                            start=True,
                            stop=True,
                            tile_position=(0, 0),
                        )
                        nc.tensor.matmul(
                            st[:, 512:1024],
                            hi,
                            qtr[64:128, icp * 512 : (icp + 1) * 512],
                            start=True,
                            stop=True,
                            tile_position=(64, 0),
                        )

                    emit_qk(0)
                    emit_qk(1)
                    for jb in range(16):
                        st = sts.pop(jb)
                        er = epool.tile([128, 1024], pvd, tag="e", name="er")
                        # first chunk: exact exp on ACT (undoes the pre-scale)
                        nc.scalar.activation(
                            er[:, 0:ACT_COLS], st[:, 0:ACT_COLS], EXP, scale=ACT_SCALE
                        )
                        # rest: fast-exp = one ADD into int16 bf16 bits
                        nc.vector.tensor_scalar_add(
                            er[:].bitcast(i16)[:, ACT_COLS:1024],
                            st[:, ACT_COLS:1024],
                            FE_ADD,
                        )
                        if jb + 2 < 16:
                            emit_qk(jb + 2)
                        for s in range(2):
                            nc.tensor.matmul(
                                o_ps[0:65, s * 512 : (s + 1) * 512],
                                v_r[:, jb * 65 : (jb + 1) * 65],
                                er[:, s * 512 : (s + 1) * 512],
                                start=(jb == 0),
                                stop=(jb == 15),
                            )
                        if h == 0 and icp == 0 and jb == 0:
                            warm_burst(16)
                        run_fillers(jb)

                    flush_fillers()
                    queue_norm(o_ps, icp, out_sb)

                    def half_dma(h=h, icp=icp, out_sb=out_sb):
                        nc.sync.dma_start(
                            out=out[h].rearrange("(b p) d -> p b d", p=128)[
                                :, icp * 8 : (icp + 1) * 8, :
                            ],
                            in_=out_sb.rearrange("p (b d) -> p b d", b=NB)[
                                :, icp * 8 : (icp + 1) * 8, :
                            ],
                        )

                    fillers.append(half_dma)

                if nxt:
                    v_r, qtr, kraw, ksw = (
                        nxt["v_r"],
                        nxt["qtr"],
                        nxt["kraw"],
                        nxt["ksw"],
                    )
                    nxt = {}

            flush_fillers()

    nc.compile()
    return nc


def _get_nc():
    if "nc" not in _cache:
        _cache["nc"] = _build()
    return _cache["nc"]


def kernel(q: np.ndarray, k: np.ndarray, v: np.ndarray) -> np.ndarray:
    from concourse.bass_utils import run_bass_kernel_spmd

    nc = _get_nc()
    qf = np.ascontiguousarray(np.asarray(q), dtype=np.float32).reshape(B * H, N, D)
    kf = np.ascontiguousarray(np.asarray(k), dtype=np.float32).reshape(B * H, N, D)
    vf = np.ascontiguousarray(np.asarray(v), dtype=np.float32).reshape(B * H, N, D)
    in_maps = [
        {
            "q": qf[c * HPC : (c + 1) * HPC],
            "k": kf[c * HPC : (c + 1) * HPC],
            "v": vf[c * HPC : (c + 1) * HPC],
        }
        for c in range(NCORES)
    ]
    r = run_bass_kernel_spmd(nc, in_maps, list(range(NCORES)))
    outs = np.concatenate([r.results[c]["out"] for c in range(NCORES)], axis=0)
    return outs.reshape(B, H, N, D).astype(np.float32)


# revision 21
# speedup vs baseline: 1.0085x; 1.0085x over previous
"""Trainium2 Bass kernel for batched multi-head softmax attention.

Problem: q,k,v [B=4, H=16, N=2048, D=64] fp32.
  out = softmax(q @ k^T / sqrt(D)) @ v   (per b,h)

Sharding: B*H = 64 head-slices, 8 per core across 8 cores. Each core
computes full attention for its 8 heads independently (no collectives).

Per-head algorithm on one core (i = query index, j = key index):
  - Load Q,K natural f32, cast to bf16 (DVE). K is pre-scaled by
    FE_MUL=128*log2(e)/8 during the cast so the QK product lands
    already in the fast-exp domain.
  - PE-transpose Q,K. The transpose of a [128, 2*64] slice lands even
    blocks on partitions 0-63 and odd blocks on partitions 64-127:
    for Q that IS the split-stream layout needed for PE row tiling
    (no duplication); for K a second GPSIMD-built tile with swapped
    halves provides each j-block on both partition halves.
  - V' = [V | 1] per j-block, bf16: the 65th PV output row accumulates
    sum_j e = the softmax denominator for free.
  - Flash-style loop: for each i-half icp (1024), for each j-block jb:
      row-tiled QK: tile (0,0) computes S'^T[j, even i] from partitions
      0-63, tile (64,0) computes S'^T[j, odd i] from partitions 64-127,
      concurrently (K=64 contraction only needs half the PE rows).
      S' = FE_MUL/8 * S_raw (pre-scaled via K).
      E: even half exact exp on ACT (scale folds the pre-scale back);
      odd half Schraudolph fast-exp on DVE, now a single tensor_scalar
      ADD: int16(S' + 16249) is the bf16 bit pattern of ~exp(S_raw/8)
      (~1.8% rms; inside the 2e-2 budget, partially cancelling in the
      softmax normalization).
      O^T += V'[jb]^T . E (bf16 accumulating matmuls, 2x512 moving;
      the ISA caps moving free size at 512).
  - O^T is PE-transposed back and scaled by 1/denominator (DVE).
"""

import numpy as np
from collections import deque

B, H, N, D = 4, 16, 2048, 64
NCORES = 8
HPC = (B * H) // NCORES  # heads per core = 8
NB = N // 128  # 16 j-blocks / i-blocks of 128
SCALE = float(D) ** -0.5

# Schraudolph fast-exp: int16 bf16-bit-pattern of exp(s) for s=S_raw/8:
#   bits = trunc(s * 128*log2(e) + (127*128 - C + 0.5))
# C=7.5 minimizes rms relative error (~1.78%); +0.5 converts the
# engine's truncating f32->i16 store into round-to-nearest. The
# multiply is folded into the K bf16 cast (FE_MUL), so the DVE op in
# the inner loop is a single ADD.
FE_MUL = 128.0 * 1.4426950408889634 * SCALE
FE_ADD = 127.0 * 128.0 - 7.5 + 0.5
ACT_SCALE = SCALE / FE_MUL  # activation sees S'=FE_MUL*S_raw; wants S_raw/8
ACT_COLS = 640  # exp split: ACT does [0:ACT_COLS), DVE fast-exp the rest

_cache = {}


def _build(hpc=HPC, qk_dt="bfloat16", pv_dt="bfloat16"):
    # note: walrus's --enable-ldw-opt (redundant LDWEIGHTS elimination)
    # would remove the repeated PV stationary loads, but the pass crashes
    # codegen (visitInstLdweights unhandled exception) on this kernel.
    import concourse.bacc as bacc
    import concourse.tile as tile
    from concourse import mybir
    from concourse.masks import make_identity

    f32 = mybir.dt.float32
    i16 = mybir.dt.int16
    qkd = getattr(mybir.dt, qk_dt)
    pvd = getattr(mybir.dt, pv_dt)
    EXP = mybir.ActivationFunctionType.Exp

    nc = bacc.Bacc("TRN2", target_bir_lowering=False, debug=False)
    q = nc.dram_tensor("q", [hpc, N, D], f32, kind="ExternalInput").ap()
    k = nc.dram_tensor("k", [hpc, N, D], f32, kind="ExternalInput").ap()
    v = nc.dram_tensor("v", [hpc, N, D], f32, kind="ExternalInput").ap()
    out = nc.dram_tensor("out", [hpc, N, D], f32, kind="ExternalOutput").ap()

    with tile.TileContext(nc) as tc:
        with (
            tc.tile_pool(name="consts", bufs=1) as consts,
            tc.tile_pool(name="stage", bufs=2) as stage,
            tc.tile_pool(name="qkt", bufs=2) as qkt,
            tc.tile_pool(name="epool", bufs=4) as epool,
            tc.tile_pool(name="osb", bufs=2) as osb,
            tc.tile_pool(name="outp", bufs=2) as outp,
            tc.tile_pool(name="stp", bufs=2, space="PSUM") as stp,
            tc.tile_pool(name="opsp", bufs=1, space="PSUM") as opsp,
            tc.tile_pool(name="tpp", bufs=2, space="PSUM") as tpp,
        ):
            warm_w = consts.tile([128, 128], mybir.dt.bfloat16)
            nc.vector.memset(warm_w[:], 0.0)
            # Preload the ACT exp table set before the first real exp
            dummy_e = consts.tile([128, 1], f32)
            nc.scalar.activation(dummy_e[:], warm_w[:, 0:1], EXP)

            ident = consts.tile([128, 128], f32)
            identb = consts.tile([128, 128], qkd)

            def emit_consts():
                # emitted after head 0's DMAs are queued so the loads start
                # as early as possible
                make_identity(nc, ident[:])
                nc.vector.tensor_copy(identb[:], ident[:])

            # PE warmup: real matmuls keep the HAM clock gate at 8/8
            def warm_burst(n):
                warm = stp.tile([128, 128], f32, tag="st", name="warm")
                for _ in range(n):
                    nc.tensor.matmul(
                        warm[:],
                        warm_w[:, 0:128],
                        warm_w[:, 0:128],
                        start=True,
                        stop=True,
                    )

            fillers = deque()

            def run_fillers(jb, njb=16):
                left = njb - jb
                kk = (len(fillers) + left - 1) // left if left > 0 else len(fillers)
                for _ in range(min(kk, len(fillers))):
                    fillers.popleft()()

            def flush_fillers():
                while fillers:
                    fillers.popleft()()

            def emit_loads(h):
                """DMA + casts for head h. Returns tiles."""
                q_nat = stage.tile([128, NB * D], f32, tag="q_nat", name="q_nat")
                nc.sync.dma_start(
                    out=q_nat.rearrange("p (b d) -> p b d", b=NB),
                    in_=q[h].rearrange("(b p) d -> p b d", p=128),
                )
                k_nat = stage.tile([128, NB * D], f32, tag="k_nat", name="k_nat")
                nc.sync.dma_start(
                    out=k_nat.rearrange("p (b d) -> p b d", b=NB),
                    in_=k[h].rearrange("(b p) d -> p b d", p=128),
                )
                q_bf = stage.tile([128, NB * D], qkd, tag="q_bf", name="q_bf")
                nc.vector.tensor_copy(q_bf[:], q_nat[:])
                # K cast folds the fast-exp multiplier
                k_bf = stage.tile([128, NB * D], qkd, tag="k_bf", name="k_bf")
                nc.vector.tensor_scalar_mul(k_bf[:], k_nat[:], FE_MUL)
                v_stage = stage.tile(
                    [128, NB * (D + 1)], f32, tag="v_stage", name="v_stage"
                )
                nc.sync.dma_start(
                    out=v_stage.rearrange("p (b e) -> p b e", b=NB)[:, :, 0:D],
                    in_=v[h].rearrange("(b p) d -> p b d", p=128),
                )
                nc.gpsimd.memset(
                    v_stage.rearrange("p (b e) -> p b e", b=NB)[:, :, D : D + 1], 1.0
                )
                v_r = stage.tile([128, NB * (D + 1)], pvd, tag="v_r", name="v_r")
                nc.vector.tensor_copy(v_r[:], v_stage[:])
                return q_bf, k_bf, v_r

            def queue_transposes(q_bf, k_bf, prologue=False):
                """Build Q^T [128,1024] (even i-blocks on parts 0-63, odd on
                64-127), K^T raw [128,1024] (even j-blocks low / odd high)
                and K^T swapped [128,1024] (odd low / even high)."""
                qtr = qkt.tile([128, N // 2], qkd, tag="qt", name="qtr")
                kraw = qkt.tile([128, N // 2], qkd, tag="kr", name="kraw")
                ksw = qkt.tile([128, N // 2], qkd, tag="ks", name="ksw")
                idx = 0
                # interleave Q and K so a partial prologue flush covers the
                # first jb iterations of both operands
                for t2 in range(NB // 2):
                    for src, dst in ((q_bf, qtr), (k_bf, kraw)):
                        idx += 1

                        def tr(src=src, dst=dst, t2=t2, idx=idx):
                            tp = (
                                stp.tile([128, 128], qkd, tag="st", name="tp")
                                if (prologue and idx % 2)
                                else tpp.tile([128, 128], qkd, tag="tp", name="tp")
                            )
                            nc.tensor.matmul(
                                tp[:],
                                src[:, t2 * 2 * D : (t2 * 2 + 2) * D],
                                identb[:, 0:128],
                                is_transpose=True,
                            )
                            cols = slice(t2 * 128, (t2 + 1) * 128)
                            nc.vector.tensor_copy(dst[:, cols], tp[:])
                            if dst is kraw:
                                # swapped-halves duplicate for the row tiles
                                if prologue:
                                    nc.scalar.copy(ksw[0:64, cols], tp[64:128, :])
                                    nc.scalar.copy(ksw[64:128, cols], tp[0:64, :])
                                else:
                                    nc.gpsimd.tensor_copy(
                                        ksw[0:64, cols], dst[64:128, cols]
                                    )
                                    nc.gpsimd.tensor_copy(
                                        ksw[64:128, cols], dst[0:64, cols]
                                    )

                        fillers.append(tr)
                return qtr, kraw, ksw

            def queue_norm(o_ps, icp, out_sb):
                """Copy O^T out of PSUM (frees the accumulator), queue the
                transpose+normalize steps as fillers. Column c of o_ps is
                query i = icp*1024 + 256*((c%512)>>7) + 128*(c>=512) + (c&127).
                The 4 t-blocks of one s-half transpose into a single PSUM
                tile, then one reciprocal + one stride-0-broadcast multiply
                normalizes all 4 blocks (out block index = icp*8 + 2t + s)."""
                o_sb = osb.tile([65, 1024], f32, tag="o_sb", name="o_sb")
                nc.vector.tensor_copy(o_sb[:, 0:512], o_ps[0:65, 0:512])
                nc.vector.tensor_copy(o_sb[:, 512:1024], o_ps[0:65, 512:1024])
                for s in range(2):
                    pt4 = tpp.tile([128, 4 * 65], f32, tag="tp", name="pt4")
                    for t in range(4):

                        def step_t(s=s, t=t, o_sb=o_sb, pt4=pt4):
                            c0 = s * 512 + t * 128
                            nc.tensor.matmul(
                                pt4[:, t * 65 : (t + 1) * 65],
                                o_sb[:, c0 : c0 + 128],
                                ident[0:65, 0:65],
                                is_transpose=True,
                            )

                        fillers.append(step_t)

                    def step_norm(s=s, icp=icp, out_sb=out_sb, pt4=pt4):
                        ptv = pt4[:].rearrange("p (t c) -> p t c", c=65)
                        rec = osb.tile([128, 4], f32, tag="rec", name="rec")
                        nc.vector.reciprocal(rec[:], ptv[:, :, 64:65].squeeze(-1))
                        ov = out_sb.rearrange(
                            "p (a t s d) -> p a t s d", a=2, t=4, s=2
                        )
                        nc.vector.tensor_tensor(
                            ov[:, icp, :, s, :],
                            ptv[:, :, 0:64],
                            rec[:].unsqueeze(-1).broadcast_to([128, 4, D]),
                            mybir.AluOpType.mult,
                        )

                    fillers.append(step_norm)

            # ---------- prologue: head 0 ----------
            q_bf, k_bf, v_r = emit_loads(0)
            emit_consts()
            qtr, kraw, ksw = queue_transposes(q_bf, k_bf, prologue=True)
            warm_burst(24)
            # flush only the transposes the first jb iterations need (Q/K
            # t2 0-3, interleaved); the rest drain as in-loop fillers
            for _ in range(8):
                fillers.popleft()()
            nxt = {}

            for h in range(hpc):
                out_sb = outp.tile([128, NB * D], f32, tag="out_sb", name="out_sb")

                for icp in range(2):
                    if icp == 1 and h + 1 < hpc:
                        nq_bf, nk_bf, nv_r = emit_loads(h + 1)
                        nqtr, nkraw, nksw = queue_transposes(nq_bf, nk_bf)
                        nxt = {"v_r": nv_r, "qtr": nqtr, "kraw": nkraw, "ksw": nksw}

                    o_ps = opsp.tile([128, 1024], f32, tag="o", name="o_ps")
                    sts = {}

                    def emit_qk(jb, icp=icp, sts=sts, qtr=qtr, kraw=kraw, ksw=ksw):
                        st = stp.tile([128, 1024], f32, tag="st", name="st")
                        sts[jb] = st
                        m = jb // 2
                        cols = slice(m * 128, (m + 1) * 128)
                        if jb % 2 == 0:
                            lo, hi = kraw[0:64, cols], ksw[64:128, cols]
                        else:
                            lo, hi = ksw[0:64, cols], kraw[64:128, cols]
                        nc.tensor.matmul(
                            st[:, 0:512],
                            lo,
                            qtr[0:64, icp * 512 : (icp + 1) * 512],
                            start=True,
                            stop=True,
                            tile_position=(0, 0),
                        )
                        nc.tensor.matmul(
                            st[:, 512:1024],
                            hi,
                            qtr[64:128, icp * 512 : (icp + 1) * 512],
                            start=True,
                            stop=True,
                            tile_position=(64, 0),
                        )

                    emit_qk(0)
                    emit_qk(1)
                    for jb in range(16):
                        st = sts.pop(jb)
                        er = epool.tile([128, 1024], pvd, tag="e", name="er")
                        # first chunk: exact exp on ACT (undoes the pre-scale)
                        nc.scalar.activation(
                            er[:, 0:ACT_COLS], st[:, 0:ACT_COLS], EXP, scale=ACT_SCALE
                        )
                        # rest: fast-exp = one ADD into int16 bf16 bits
                        nc.vector.tensor_scalar_add(
                            er[:].bitcast(i16)[:, ACT_COLS:1024],
                            st[:, ACT_COLS:1024],
                            FE_ADD,
                        )
                        if jb + 2 < 16:
                            emit_qk(jb + 2)
                        for s in range(2):
                            nc.tensor.matmul(
                                o_ps[0:65, s * 512 : (s + 1) * 512],
                                v_r[:, jb * 65 : (jb + 1) * 65],
                                er[:, s * 512 : (s + 1) * 512],
                                start=(jb == 0),
                                stop=(jb == 15),
                            )
                        if h == 0 and icp == 0 and jb == 0:
                            warm_burst(16)
                        run_fillers(jb)

                    flush_fillers()
                    queue_norm(o_ps, icp, out_sb)

                    def half_dma(h=h, icp=icp, out_sb=out_sb):
                        nc.sync.dma_start(
                            out=out[h].rearrange("(b p) d -> p b d", p=128)[
                                :, icp * 8 : (icp + 1) * 8, :
                            ],
                            in_=out_sb.rearrange("p (b d) -> p b d", b=NB)[
                                :, icp * 8 : (icp + 1) * 8, :
                            ],
                        )

                    fillers.append(half_dma)

                if nxt:
                    v_r, qtr, kraw, ksw = (
                        nxt["v_r"],
                        nxt["qtr"],
                        nxt["kraw"],
                        nxt["ksw"],
                    )
                    nxt = {}

            flush_fillers()

    nc.compile()
    return nc


def _get_nc():
    if "nc" not in _cache:
        _cache["nc"] = _build()
    return _cache["nc"]


def kernel(q: np.ndarray, k: np.ndarray, v: np.ndarray) -> np.ndarray:
    from concourse.bass_utils import run_bass_kernel_spmd

    nc = _get_nc()
    qf = np.ascontiguousarray(np.asarray(q), dtype=np.float32).reshape(B * H, N, D)
    kf = np.ascontiguousarray(np.asarray(k), dtype=np.float32).reshape(B * H, N, D)
    vf = np.ascontiguousarray(np.asarray(v), dtype=np.float32).reshape(B * H, N, D)
    in_maps = [
        {
            "q": qf[c * HPC : (c + 1) * HPC],
            "k": kf[c * HPC : (c + 1) * HPC],
            "v": vf[c * HPC : (c + 1) * HPC],
        }
        for c in range(NCORES)
    ]
    r = run_bass_kernel_spmd(nc, in_maps, list(range(NCORES)))
    outs = np.concatenate([r.results[c]["out"] for c in range(NCORES)], axis=0)
    return outs.reshape(B, H, N, D).astype(np.float32)
